# revision 13
# baseline (speedup 1.0000x reference)
"""v3: causal self-attention, 8 trn2 cores.

Key additions over v2:
- S matmuls (64-wide contraction) are emitted for HEAD PAIRS with
  alternating PE row-quadrants (qrow 0 / 64). The PE runs the two
  64-row tile streams concurrently: measured 112ns vs 219ns per 512-col
  matmul, i.e. ~2x throughput for all S work.
- The softmax ones-column sits at V index 0, so the PV output row 0 is
  the rowsum at PARTITION 0 -- the fast DVE reciprocal (which requires
  base partition 0) reads it directly; no ACT shift-copy.
- 1/rowsum is broadcast across partitions by the GpSimd ucode
  partition_broadcast (attn library) instead of a PE outer product:
  frees a PSUM bank and PE cycles.
- Output projection accumulates in single-bank [128,512] PSUM halves.
- PSUM: stps 3x2 banks + ot 1 + proj 1 = 8.
"""

from contextlib import ExitStack

import ml_dtypes
import numpy as np

import concourse.bass as bass  # noqa: F401
import concourse.mybir as mybir
import concourse.tile as tile
from concourse import bacc
from concourse.bass_utils import run_bass_kernel_spmd

B, T, C, NH = 4, 2048, 1024, 16
HD = 64
NCORES = 8
HPC = NH // 2
DH = HPC * HD
TS = T // 512
NT = T // 128
NC_CHUNKS = C // 128
PTW = 2048 * (TS - 1) + 1280

F32 = mybir.dt.float32
F16 = mybir.dt.float16
EXP = mybir.ActivationFunctionType.Exp

TRACE = False
TRACE_KW = {}
LAST_RESULT = None

_nc_cache = None


def _span_blocks(s):
    out = []
    off = 0
    for j in range(4 * s + 4):
        qo = max(s * 512, j * 128)
        w = (s + 1) * 512 - qo
        out.append((j, qo, w, off))
        off += w
    return out


def _span_pairs(s):
    blocks = _span_blocks(s)
    pairs = []
    for i in range(0, len(blocks), 2):
        chunk = blocks[i:i + 2]
        halves = []
        off_in_pair = 0
        for (j, qo, w, poff) in chunk:
            halves.append((j, qo, w, off_in_pair, j * 128 >= s * 512))
            off_in_pair += w
        pairs.append((halves, off_in_pair, chunk[0][3]))
    return pairs


def _build():
    nc = bacc.Bacc("TRN2", target_bir_lowering=False)

    xT_d = nc.dram_tensor("xT", [128, NC_CHUNKS, T], F16, kind="ExternalInput")
    wqk_d = nc.dram_tensor("wqk", [128, 8, NC_CHUNKS, 128], F16, kind="ExternalInput")
    wv_d = nc.dram_tensor("wv", [128, NC_CHUNKS, DH], F16, kind="ExternalInput")
    wp_d = nc.dram_tensor("wp", [128, DH // 128, C], F16, kind="ExternalInput")
    mask01_d = nc.dram_tensor("mask01", [128, 128], F16, kind="ExternalInput")
    out_d = nc.dram_tensor("out", [T, C], F32, kind="ExternalOutput")

    with tile.TileContext(nc) as tc, ExitStack() as ctx:
        const = ctx.enter_context(tc.tile_pool(name="const", bufs=1))
        persist = ctx.enter_context(tc.tile_pool(name="persist", bufs=1))

        mask01 = const.tile([128, 128], F16)
        nc.sync.dma_start(mask01[:], mask01_d[:])

        qk_sb = [persist.tile([128, T], F16, tag=f"qk{i}", name=f"qk{i}")
                 for i in range(8)]
        # V layout: [T-tile part, head, 65] with the ones-column at idx 64
        v_sb = [persist.tile([128, HPC, 65], F16, tag=f"v{i}", name=f"v{i}")
                for i in range(NT)]
        wp_all = persist.tile([128, DH // 128, C], F16, tag="wp", name="wp")
        wqk_all = persist.tile([128, 8, NC_CHUNKS, 128], F16, tag="wqk",
                               name="wqk")
        wv_all = persist.tile([128, NC_CHUNKS, DH], F16, tag="wv", name="wv")
        # 4 pt buffers: head pair H writes 2 while pair H-1's 2 are read
        pt4 = [persist.tile([128, PTW], F16, tag=f"pt{p}", name=f"pt{p}")
               for p in range(4)]

        # split warmup loads across the two HWDGE queues (SP + Act)
        nc.scalar.dma_start(wqk_all[:], wqk_d[:])
        nc.scalar.dma_start(wv_all[:], wv_d[:])
        nc.scalar.dma_start(wp_all[:], wp_d[:])
        for t in range(NT):
            nc.gpsimd.memset(v_sb[t][:, :, 64], 1.0)

        with tc.tile_pool(name="xT", bufs=2) as xpool, \
             tc.tile_pool(name="yts", bufs=2) as ytspool, \
             tc.tile_pool(name="otsb", bufs=1) as otsbpool, \
             tc.tile_pool(name="small", bufs=2) as small, \
             tc.tile_pool(name="outst", bufs=2) as outst, \
             tc.tile_pool(name="stps", bufs=3, space="PSUM") as stps, \
             tc.tile_pool(name="otps", bufs=1, space="PSUM") as otps, \
             tc.tile_pool(name="pps", bufs=1, space="PSUM") as pps:

            xT_cur = [None]

            def load_x(s):
                xt = xpool.tile([128, NC_CHUNKS, 512], F16, tag="x",
                                name="x")
                nc.sync.dma_start(
                    xt[:], xT_d[:, :, s * 512:(s + 1) * 512])
                xT_cur[0] = xt

            def qkv_chunks(s):
                """Yield emission closures for qkv work of span s."""
                items = []
                for fp in range(4):
                    def emit_qk(fp=fp, s=s):
                        ps = stps.tile([128, 1024], F32, tag="st", name="st")
                        for half in range(2):
                            ft = 2 * fp + half
                            hs = slice(half * 512, (half + 1) * 512)
                            for c in range(NC_CHUNKS):
                                nc.tensor.matmul(
                                    ps[:, hs], wqk_all[:, ft, c, :],
                                    xT_cur[0][:, c, :],
                                    start=(c == 0), stop=(c == NC_CHUNKS - 1),
                                    skip_group_check=True)
                        for half in range(2):
                            hs = slice(half * 512, (half + 1) * 512)
                            nc.vector.tensor_copy(
                                qk_sb[2 * fp + half][:, s * 512:(s + 1) * 512],
                                ps[:, hs])
                    items.append(emit_qk)
                for vp_ in range(2):
                    def emit_v(vp_=vp_, s=s):
                        ps = stps.tile([128, 1024], F32, tag="st", name="st")
                        for half in range(2):
                            t4 = 2 * vp_ + half
                            hs = slice(half * 512, (half + 1) * 512)
                            for c in range(NC_CHUNKS):
                                nc.tensor.matmul(
                                    ps[:, hs],
                                    xT_cur[0][:, c,
                                              t4 * 128:(t4 + 1) * 128],
                                    wv_all[:, c, :],
                                    start=(c == 0), stop=(c == NC_CHUNKS - 1),
                                    skip_group_check=True)
                        for half in range(2):
                            t = 4 * s + 2 * vp_ + half
                            hs = slice(half * 512, (half + 1) * 512)
                            nc.vector.tensor_copy(
                                v_sb[t][:, :, 0:64],
                                ps[:, hs].rearrange("p (h d) -> p h d", h=HPC))
                    items.append(emit_v)
                return items

            def emit_s_pair_2h(s, h0, p0, p1, pair):
                """S for heads h0 (quad row 0) and h0+1 (quad row 64),
                matmuls interleaved for concurrent quadrant streams."""
                halves, pair_w, pt_off = pair
                qch = h0 // 2
                qT = qk_sb[qch]
                kT = qk_sb[4 + qch]
                ps0 = stps.tile([128, 1024], F32, tag="st", name="st")
                ps1 = stps.tile([128, 1024], F32, tag="st", name="st")
                for (j, qo, w, oip, diag) in halves:
                    ks = slice(j * 128, (j + 1) * 128)
                    nc.tensor.matmul(
                        ps0[:, oip:oip + w], kT[0:64, ks], qT[0:64, qo:qo + w],
                        start=True, stop=True, skip_group_check=True)
                    nc.tensor.matmul(
                        ps1[:, oip:oip + w], kT[64:128, ks],
                        qT[64:128, qo:qo + w],
                        start=True, stop=True, skip_group_check=True)
                nc.scalar.activation(
                    pt4[p0][:, pt_off:pt_off + pair_w], ps0[:, 0:pair_w], EXP)
                nc.scalar.activation(
                    pt4[p1][:, pt_off:pt_off + pair_w], ps1[:, 0:pair_w], EXP)
                for (j, qo, w, oip, diag) in halves:
                    if diag:
                        for p in (p0, p1):
                            sl = slice(pt_off + oip, pt_off + oip + 128)
                            nc.vector.tensor_tensor(
                                pt4[p][:, sl], pt4[p][:, sl], mask01[:],
                                mybir.AluOpType.mult)

            def emit_pv_blocks(s, h, p, blocks, ot, jmax):
                for (j, qo, w, poff) in blocks:
                    rel = qo - s * 512
                    nc.tensor.matmul(
                        ot[0:65, rel:rel + w],
                        v_sb[j][:, h, :], pt4[p][:, poff:poff + w],
                        start=(j == 0), stop=(j == jmax),
                        skip_group_check=True)

            def norm_head(h, yts, otsb):
                qch, qrow = h // 2, 64 * (h % 2)
                rs = small.tile([1, 512], F32, tag="rs", name="rs")
                nc.scalar.copy(rs[:], otsb[h][64:65, :])
                r32 = small.tile([1, 512], F32, tag="r32", name="r32")
                nc.vector.reciprocal_approx_fast(r32[:], rs[:])
                rinv = small.tile([1, 512], F16, tag="rinv", name="rinv",
                                  bufs=4)
                nc.vector.tensor_copy(rinv[:], r32[:])
                rbs = small.tile([64, 512], F16, tag="rbs", name="rbs",
                                 bufs=4)
                nc.gpsimd.partition_broadcast(rbs[:], rinv[:], channels=64)
                nc.vector.tensor_tensor(
                    yts[qch][qrow:qrow + 64, :], otsb[h][0:64, :],
                    rbs[:], mybir.AluOpType.mult)

            def proj_tt(s, tt_rel, yts):
                tt = s * 4 + tt_rel
                for n in range(2):
                    po = pps.tile([128, 512], F32, tag="pp", name="pp")
                    for c in range(DH // 128):
                        nc.tensor.matmul(
                            po[:],
                            yts[c][:, tt_rel * 128:(tt_rel + 1) * 128],
                            wp_all[:, c, n * 512:(n + 1) * 512],
                            start=(c == 0), stop=(c == DH // 128 - 1))
                    ob = outst.tile([128, 512], F32, tag="ob", name="ob")
                    nc.vector.tensor_copy(ob[:], po[:])
                    nc.sync.dma_start(
                        out_d[tt * 128:(tt + 1) * 128,
                              n * 512:(n + 1) * 512], ob[:])

            prev = None  # (s, yts, otsb list)
            load_x(0)
            for it in qkv_chunks(0):
                it()  # qkv of span 0 runs before any attention
            for s in range(TS):
                pairs = _span_pairs(s)
                blocks = _span_blocks(s)
                jmax = 4 * s + 3
                if s < TS - 1:
                    load_x(s + 1)
                # qkv(s+1) chunks are emitted inside the H==3 phase below
                next_qkv = qkv_chunks(s + 1) if s < TS - 1 else []
                yts_cur = [ytspool.tile([128, 512], F16, tag=f"yts{i}",
                                        name=f"yts{i}")
                           for i in range(DH // 128)]
                otsb_cur = [None] * HPC

                # norms of the previous span only need prev-span data:
                # emit them up front so proj can interleave anywhere later
                if prev is not None:
                    ps_, yts_p, otsb_p = prev
                    for h in range(HPC):
                        norm_head(h, yts_p, otsb_p)
                for pair in pairs:
                    emit_s_pair_2h(s, 0, 0, 1, pair)

                for H in range(4):
                    h0 = 2 * H
                    pb = 2 * (H % 2)         # pt slots of pair H
                    nb = 2 * ((H + 1) % 2)   # pt slots of pair H+1
                    np_pairs = pairs if H < 3 else []
                    # PV for heads h0, h0+1 interleaved with S of pair H+1
                    # (or, at H==3, with qkv of span s+1)
                    ot0 = otps.tile([128, 512], F32, tag="ot", name="ot")
                    bi = 0
                    qn = 0
                    nsteps = max(len(np_pairs), (len(blocks) + 3) // 4)
                    for k in range(nsteps):
                        if k < len(np_pairs):
                            emit_s_pair_2h(s, h0 + 2, nb, nb + 1,
                                           np_pairs[k])
                        elif H == 3 and qn < len(next_qkv):
                            next_qkv[qn](); qn += 1
                        chunk = blocks[bi:bi + 4]
                        bi += 4
                        emit_pv_blocks(s, h0, pb, chunk, ot0, jmax)
                    if bi < len(blocks):
                        emit_pv_blocks(s, h0, pb, blocks[bi:], ot0, jmax)
                    ob0 = otsbpool.tile([65, 512], F32, tag=f"otsb{h0}",
                                        name=f"otsb{h0}")
                    nc.vector.tensor_copy(ob0[:], ot0[0:65, :])
                    otsb_cur[h0] = ob0
                    if s == TS - 1:
                        norm_head(h0, yts_cur, otsb_cur)

                    ot1 = otps.tile([128, 512], F32, tag="ot", name="ot")
                    bi = 0
                    for k in range((len(blocks) + 3) // 4):
                        if H == 3 and qn < len(next_qkv):
                            next_qkv[qn](); qn += 1
                        chunk = blocks[bi:bi + 4]
                        bi += 4
                        emit_pv_blocks(s, h0 + 1, pb + 1, chunk, ot1, jmax)
                    if bi < len(blocks):
                        emit_pv_blocks(s, h0 + 1, pb + 1, blocks[bi:],
                                       ot1, jmax)
                    while H == 3 and qn < len(next_qkv):
                        next_qkv[qn](); qn += 1
                    if prev is not None:
                        ps_, yts_p, otsb_p = prev
                        proj_tt(ps_, H, yts_p)
                    ob1 = otsbpool.tile([65, 512], F32, tag=f"otsb{h0+1}",
                                        name=f"otsb{h0+1}")
                    nc.vector.tensor_copy(ob1[:], ot1[0:65, :])
                    otsb_cur[h0 + 1] = ob1
                    if s == TS - 1:
                        norm_head(h0 + 1, yts_cur, otsb_cur)

                prev = (s, yts_cur, otsb_cur)

            ps_, yts_p, otsb_p = prev
            for tt_rel in range(4):
                proj_tt(ps_, tt_rel, yts_p)

    nc.compile()
    return nc


def _get_nc():
    global _nc_cache
    if _nc_cache is None:
        _nc_cache = _build()
    return _nc_cache


def kernel(x, w_attn, b_attn, w_proj, b_proj):
    x = np.asarray(x, dtype=np.float32)
    w_attn = np.asarray(w_attn, dtype=np.float32)
    b_attn = np.asarray(b_attn, dtype=np.float32)
    w_proj = np.asarray(w_proj, dtype=np.float32)
    b_proj = np.asarray(b_proj, dtype=np.float32)

    nc = _get_nc()

    ii = np.arange(128)
    mask01 = (ii[:, None] <= ii[None, :]).astype(np.float16)
    f16 = np.float16

    in_maps = []
    for core in range(NCORES):
        b, g = core // 2, core % 2
        fs = slice(g * DH, (g + 1) * DH)
        wq = w_attn[:, fs] * 0.125
        wk = w_attn[:, C + g * DH: C + (g + 1) * DH]
        wv = w_attn[:, 2 * C + g * DH: 2 * C + (g + 1) * DH]
        w2 = np.concatenate([wq, wk], axis=1)
        wqk = np.ascontiguousarray(
            w2.reshape(NC_CHUNKS, 128, 8, 128).transpose(1, 2, 0, 3))
        xTb = np.ascontiguousarray(
            x[b].T.reshape(NC_CHUNKS, 128, T).transpose(1, 0, 2))
        wvb = np.ascontiguousarray(
            wv.reshape(NC_CHUNKS, 128, DH).transpose(1, 0, 2))
        wpb = np.ascontiguousarray(
            w_proj[fs, :].reshape(DH // 128, 128, C).transpose(1, 0, 2))
        in_maps.append({
            "xT": xTb.astype(f16),
            "wqk": wqk.astype(f16),
            "wv": wvb.astype(f16),
            "wp": wpb.astype(f16),
            "mask01": mask01,
        })

    global LAST_RESULT
    res = run_bass_kernel_spmd(
        nc, in_maps, core_ids=list(range(NCORES)),
        trace=TRACE, **(TRACE_KW if TRACE else {}))
    LAST_RESULT = res

    corr = b_proj + b_attn[2 * C:3 * C] @ w_proj
    out = np.empty((B, T, C), dtype=np.float32)
    for b in range(B):
        out[b] = res.results[2 * b]["out"] + res.results[2 * b + 1]["out"] + corr
    return out


# revision 14
# speedup vs baseline: 1.0134x; 1.0134x over previous
"""v3: causal self-attention, 8 trn2 cores.

Key additions over v2:
- S matmuls (64-wide contraction) are emitted for HEAD PAIRS with
  alternating PE row-quadrants (qrow 0 / 64). The PE runs the two
  64-row tile streams concurrently: measured 112ns vs 219ns per 512-col
  matmul, i.e. ~2x throughput for all S work.
- The softmax ones-column sits at V index 0, so the PV output row 0 is
  the rowsum at PARTITION 0 -- the fast DVE reciprocal (which requires
  base partition 0) reads it directly; no ACT shift-copy.
- 1/rowsum is broadcast across partitions by the GpSimd ucode
  partition_broadcast (attn library) instead of a PE outer product:
  frees a PSUM bank and PE cycles.
- Output projection accumulates in single-bank [128,512] PSUM halves.
- PSUM: stps 3x2 banks + ot 1 + proj 1 = 8.
"""

from contextlib import ExitStack

import ml_dtypes
import numpy as np

import concourse.bass as bass  # noqa: F401
import concourse.mybir as mybir
import concourse.tile as tile
from concourse import bacc
from concourse.bass_utils import run_bass_kernel_spmd

B, T, C, NH = 4, 2048, 1024, 16
HD = 64
NCORES = 8
HPC = NH // 2
DH = HPC * HD
TS = T // 512
NT = T // 128
NC_CHUNKS = C // 128
PTW = 2048 * (TS - 1) + 1280

F32 = mybir.dt.float32
F16 = mybir.dt.float16
EXP = mybir.ActivationFunctionType.Exp

TRACE = False
TRACE_KW = {}
LAST_RESULT = None

_nc_cache = None


def _span_blocks(s):
    out = []
    off = 0
    for j in range(4 * s + 4):
        qo = max(s * 512, j * 128)
        w = (s + 1) * 512 - qo
        out.append((j, qo, w, off))
        off += w
    return out


def _span_pairs(s):
    blocks = _span_blocks(s)
    pairs = []
    for i in range(0, len(blocks), 2):
        chunk = blocks[i:i + 2]
        halves = []
        off_in_pair = 0
        for (j, qo, w, poff) in chunk:
            halves.append((j, qo, w, off_in_pair, j * 128 >= s * 512))
            off_in_pair += w
        pairs.append((halves, off_in_pair, chunk[0][3]))
    return pairs


def _build():
    nc = bacc.Bacc("TRN2", target_bir_lowering=False)

    xT_d = nc.dram_tensor("xT", [128, NC_CHUNKS, T], F16, kind="ExternalInput")
    wqk_d = nc.dram_tensor("wqk", [128, 8, NC_CHUNKS, 128], F16, kind="ExternalInput")
    wv_d = nc.dram_tensor("wv", [128, NC_CHUNKS, DH], F16, kind="ExternalInput")
    wp_d = nc.dram_tensor("wp", [128, DH // 128, C], F16, kind="ExternalInput")
    mask01_d = nc.dram_tensor("mask01", [128, 128], F16, kind="ExternalInput")
    out_d = nc.dram_tensor("out", [T, C], F32, kind="ExternalOutput")

    with tile.TileContext(nc) as tc, ExitStack() as ctx:
        const = ctx.enter_context(tc.tile_pool(name="const", bufs=1))
        persist = ctx.enter_context(tc.tile_pool(name="persist", bufs=1))

        mask01 = const.tile([128, 128], F16)
        nc.sync.dma_start(mask01[:], mask01_d[:])

        qk_sb = [persist.tile([128, T], F16, tag=f"qk{i}", name=f"qk{i}")
                 for i in range(8)]
        # V layout: [T-tile part, head, 65] with the ones-column at idx 64
        v_sb = [persist.tile([128, HPC, 65], F16, tag=f"v{i}", name=f"v{i}")
                for i in range(NT)]
        wp_all = persist.tile([128, DH // 128, C], F16, tag="wp", name="wp")
        wqk_all = persist.tile([128, 8, NC_CHUNKS, 128], F16, tag="wqk",
                               name="wqk")
        wv_all = persist.tile([128, NC_CHUNKS, DH], F16, tag="wv", name="wv")
        # 4 pt buffers: head pair H writes 2 while pair H-1's 2 are read
        pt4 = [persist.tile([128, PTW], F16, tag=f"pt{p}", name=f"pt{p}")
               for p in range(4)]

        # split warmup loads across the two HWDGE queues (SP + Act)
        nc.scalar.dma_start(wqk_all[:], wqk_d[:])
        nc.scalar.dma_start(wv_all[:], wv_d[:])
        nc.scalar.dma_start(wp_all[:], wp_d[:])
        for t in range(NT):
            nc.gpsimd.memset(v_sb[t][:, :, 64], 1.0)

        with tc.tile_pool(name="xT", bufs=2) as xpool, \
             tc.tile_pool(name="yts", bufs=2) as ytspool, \
             tc.tile_pool(name="otsb", bufs=1) as otsbpool, \
             tc.tile_pool(name="small", bufs=2) as small, \
             tc.tile_pool(name="outst", bufs=2) as outst, \
             tc.tile_pool(name="stps", bufs=2, space="PSUM") as stps, \
             tc.tile_pool(name="otps", bufs=2, space="PSUM") as otps, \
             tc.tile_pool(name="pps", bufs=2, space="PSUM") as pps:

            xT_cur = [None]

            def load_x(s):
                xt = xpool.tile([128, NC_CHUNKS, 512], F16, tag="x",
                                name="x")
                nc.sync.dma_start(
                    xt[:], xT_d[:, :, s * 512:(s + 1) * 512])
                xT_cur[0] = xt

            def qkv_chunks(s):
                """Yield emission closures for qkv work of span s."""
                items = []
                for fp in range(4):
                    def emit_qk(fp=fp, s=s):
                        ps = stps.tile([128, 1024], F32, tag="st", name="st")
                        for half in range(2):
                            ft = 2 * fp + half
                            hs = slice(half * 512, (half + 1) * 512)
                            for c in range(NC_CHUNKS):
                                nc.tensor.matmul(
                                    ps[:, hs], wqk_all[:, ft, c, :],
                                    xT_cur[0][:, c, :],
                                    start=(c == 0), stop=(c == NC_CHUNKS - 1),
                                    skip_group_check=True)
                        for half in range(2):
                            hs = slice(half * 512, (half + 1) * 512)
                            nc.vector.tensor_copy(
                                qk_sb[2 * fp + half][:, s * 512:(s + 1) * 512],
                                ps[:, hs])
                    items.append(emit_qk)
                for vp_ in range(2):
                    def emit_v(vp_=vp_, s=s):
                        ps = stps.tile([128, 1024], F32, tag="st", name="st")
                        for half in range(2):
                            t4 = 2 * vp_ + half
                            hs = slice(half * 512, (half + 1) * 512)
                            for c in range(NC_CHUNKS):
                                nc.tensor.matmul(
                                    ps[:, hs],
                                    xT_cur[0][:, c,
                                              t4 * 128:(t4 + 1) * 128],
                                    wv_all[:, c, :],
                                    start=(c == 0), stop=(c == NC_CHUNKS - 1),
                                    skip_group_check=True)
                        for half in range(2):
                            t = 4 * s + 2 * vp_ + half
                            hs = slice(half * 512, (half + 1) * 512)
                            nc.vector.tensor_copy(
                                v_sb[t][:, :, 0:64],
                                ps[:, hs].rearrange("p (h d) -> p h d", h=HPC))
                    items.append(emit_v)
                return items

            def emit_s_pair_2h(s, h0, p0, p1, pair):
                """S for heads h0 (quad row 0) and h0+1 (quad row 64),
                matmuls interleaved for concurrent quadrant streams."""
                halves, pair_w, pt_off = pair
                qch = h0 // 2
                qT = qk_sb[qch]
                kT = qk_sb[4 + qch]
                ps0 = stps.tile([128, 1024], F32, tag="st", name="st")
                ps1 = stps.tile([128, 1024], F32, tag="st", name="st")
                for (j, qo, w, oip, diag) in halves:
                    ks = slice(j * 128, (j + 1) * 128)
                    nc.tensor.matmul(
                        ps0[:, oip:oip + w], kT[0:64, ks], qT[0:64, qo:qo + w],
                        start=True, stop=True, skip_group_check=True)
                    nc.tensor.matmul(
                        ps1[:, oip:oip + w], kT[64:128, ks],
                        qT[64:128, qo:qo + w],
                        start=True, stop=True, skip_group_check=True)
                nc.scalar.activation(
                    pt4[p0][:, pt_off:pt_off + pair_w], ps0[:, 0:pair_w], EXP)
                nc.scalar.activation(
                    pt4[p1][:, pt_off:pt_off + pair_w], ps1[:, 0:pair_w], EXP)
                for (j, qo, w, oip, diag) in halves:
                    if diag:
                        for p in (p0, p1):
                            sl = slice(pt_off + oip, pt_off + oip + 128)
                            nc.vector.tensor_tensor(
                                pt4[p][:, sl], pt4[p][:, sl], mask01[:],
                                mybir.AluOpType.mult)

            def emit_pv_blocks(s, h, p, blocks, ot, jmax):
                for (j, qo, w, poff) in blocks:
                    rel = qo - s * 512
                    nc.tensor.matmul(
                        ot[0:65, rel:rel + w],
                        v_sb[j][:, h, :], pt4[p][:, poff:poff + w],
                        start=(j == 0), stop=(j == jmax),
                        skip_group_check=True)

            def norm_head(h, yts, otsb):
                qch, qrow = h // 2, 64 * (h % 2)
                rs = small.tile([1, 512], F32, tag="rs", name="rs")
                nc.scalar.copy(rs[:], otsb[h][64:65, :])
                r32 = small.tile([1, 512], F32, tag="r32", name="r32")
                nc.vector.reciprocal_approx_fast(r32[:], rs[:])
                rinv = small.tile([1, 512], F16, tag="rinv", name="rinv",
                                  bufs=4)
                nc.vector.tensor_copy(rinv[:], r32[:])
                rbs = small.tile([64, 512], F16, tag="rbs", name="rbs",
                                 bufs=4)
                nc.gpsimd.partition_broadcast(rbs[:], rinv[:], channels=64)
                nc.vector.tensor_tensor(
                    yts[qch][qrow:qrow + 64, :], otsb[h][0:64, :],
                    rbs[:], mybir.AluOpType.mult)

            def proj_tt(s, tt_rel, yts):
                tt = s * 4 + tt_rel
                for n in range(2):
                    po = pps.tile([128, 512], F32, tag="pp", name="pp")
                    for c in range(DH // 128):
                        nc.tensor.matmul(
                            po[:],
                            yts[c][:, tt_rel * 128:(tt_rel + 1) * 128],
                            wp_all[:, c, n * 512:(n + 1) * 512],
                            start=(c == 0), stop=(c == DH // 128 - 1))
                    ob = outst.tile([128, 512], F32, tag="ob", name="ob")
                    nc.vector.tensor_copy(ob[:], po[:])
                    nc.sync.dma_start(
                        out_d[tt * 128:(tt + 1) * 128,
                              n * 512:(n + 1) * 512], ob[:])

            prev = None  # (s, yts, otsb list)
            load_x(0)
            for it in qkv_chunks(0):
                it()  # qkv of span 0 runs before any attention
            for s in range(TS):
                pairs = _span_pairs(s)
                blocks = _span_blocks(s)
                jmax = 4 * s + 3
                if s < TS - 1:
                    load_x(s + 1)
                # qkv(s+1) chunks are emitted inside the H==3 phase below
                next_qkv = qkv_chunks(s + 1) if s < TS - 1 else []
                yts_cur = [ytspool.tile([128, 512], F16, tag=f"yts{i}",
                                        name=f"yts{i}")
                           for i in range(DH // 128)]
                otsb_cur = [None] * HPC

                # norms of the previous span only need prev-span data:
                # emit them up front so proj can interleave anywhere later
                if prev is not None:
                    ps_, yts_p, otsb_p = prev
                    for h in range(HPC):
                        norm_head(h, yts_p, otsb_p)
                for pair in pairs:
                    emit_s_pair_2h(s, 0, 0, 1, pair)

                for H in range(4):
                    h0 = 2 * H
                    pb = 2 * (H % 2)         # pt slots of pair H
                    nb = 2 * ((H + 1) % 2)   # pt slots of pair H+1
                    np_pairs = pairs if H < 3 else []
                    # PV for heads h0, h0+1 interleaved with S of pair H+1
                    # (or, at H==3, with qkv of span s+1)
                    ot0 = otps.tile([128, 512], F32, tag="ot", name="ot")
                    bi = 0
                    qn = 0
                    nsteps = max(len(np_pairs), (len(blocks) + 3) // 4)
                    for k in range(nsteps):
                        if k < len(np_pairs):
                            emit_s_pair_2h(s, h0 + 2, nb, nb + 1,
                                           np_pairs[k])
                        elif H == 3 and qn < len(next_qkv):
                            next_qkv[qn](); qn += 1
                        chunk = blocks[bi:bi + 4]
                        bi += 4
                        emit_pv_blocks(s, h0, pb, chunk, ot0, jmax)
                    if bi < len(blocks):
                        emit_pv_blocks(s, h0, pb, blocks[bi:], ot0, jmax)
                    ob0 = otsbpool.tile([65, 512], F32, tag=f"otsb{h0}",
                                        name=f"otsb{h0}")
                    nc.vector.tensor_copy(ob0[:], ot0[0:65, :])
                    otsb_cur[h0] = ob0
                    if s == TS - 1:
                        norm_head(h0, yts_cur, otsb_cur)

                    ot1 = otps.tile([128, 512], F32, tag="ot", name="ot")
                    bi = 0
                    for k in range((len(blocks) + 3) // 4):
                        if H == 3 and qn < len(next_qkv):
                            next_qkv[qn](); qn += 1
                        chunk = blocks[bi:bi + 4]
                        bi += 4
                        emit_pv_blocks(s, h0 + 1, pb + 1, chunk, ot1, jmax)
                    if bi < len(blocks):
                        emit_pv_blocks(s, h0 + 1, pb + 1, blocks[bi:],
                                       ot1, jmax)
                    while H == 3 and qn < len(next_qkv):
                        next_qkv[qn](); qn += 1
                    if prev is not None:
                        ps_, yts_p, otsb_p = prev
                        proj_tt(ps_, H, yts_p)
                    ob1 = otsbpool.tile([65, 512], F32, tag=f"otsb{h0+1}",
                                        name=f"otsb{h0+1}")
                    nc.vector.tensor_copy(ob1[:], ot1[0:65, :])
                    otsb_cur[h0 + 1] = ob1
                    if s == TS - 1:
                        norm_head(h0 + 1, yts_cur, otsb_cur)

                prev = (s, yts_cur, otsb_cur)

            ps_, yts_p, otsb_p = prev
            for tt_rel in range(4):
                proj_tt(ps_, tt_rel, yts_p)

    nc.compile()
    return nc


def _get_nc():
    global _nc_cache
    if _nc_cache is None:
        _nc_cache = _build()
    return _nc_cache


def kernel(x, w_attn, b_attn, w_proj, b_proj):
    x = np.asarray(x, dtype=np.float32)
    w_attn = np.asarray(w_attn, dtype=np.float32)
    b_attn = np.asarray(b_attn, dtype=np.float32)
    w_proj = np.asarray(w_proj, dtype=np.float32)
    b_proj = np.asarray(b_proj, dtype=np.float32)

    nc = _get_nc()

    ii = np.arange(128)
    mask01 = (ii[:, None] <= ii[None, :]).astype(np.float16)
    f16 = np.float16

    in_maps = []
    for core in range(NCORES):
        b, g = core // 2, core % 2
        fs = slice(g * DH, (g + 1) * DH)
        wq = w_attn[:, fs] * 0.125
        wk = w_attn[:, C + g * DH: C + (g + 1) * DH]
        wv = w_attn[:, 2 * C + g * DH: 2 * C + (g + 1) * DH]
        w2 = np.concatenate([wq, wk], axis=1)
        wqk = np.ascontiguousarray(
            w2.reshape(NC_CHUNKS, 128, 8, 128).transpose(1, 2, 0, 3))
        xTb = np.ascontiguousarray(
            x[b].T.reshape(NC_CHUNKS, 128, T).transpose(1, 0, 2))
        wvb = np.ascontiguousarray(
            wv.reshape(NC_CHUNKS, 128, DH).transpose(1, 0, 2))
        wpb = np.ascontiguousarray(
            w_proj[fs, :].reshape(DH // 128, 128, C).transpose(1, 0, 2))
        in_maps.append({
            "xT": xTb.astype(f16),
            "wqk": wqk.astype(f16),
            "wv": wvb.astype(f16),
            "wp": wpb.astype(f16),
            "mask01": mask01,
        })

    global LAST_RESULT
    res = run_bass_kernel_spmd(
        nc, in_maps, core_ids=list(range(NCORES)),
        trace=TRACE, **(TRACE_KW if TRACE else {}))
    LAST_RESULT = res

    corr = b_proj + b_attn[2 * C:3 * C] @ w_proj
    out = np.empty((B, T, C), dtype=np.float32)
    for b in range(B):
        out[b] = res.results[2 * b]["out"] + res.results[2 * b + 1]["out"] + corr
    return out
